# revision 56
# baseline (speedup 1.0000x reference)
"""GAT layer kernel for Trainium2, 8-core row-parallel SPMD.

Math (reference):
    agg  = (A @ X) @ W + b
    si   = agg @ phi[:F];  sj = agg @ phi[F:]
    H    = si[:,None] + sj[None,:];  mask = (A + I) != 0
    attn = softmax(where(mask, H, -inf), axis=-1)
    out  = relu(attn @ agg)

Key identity: si[i] cancels in the row softmax, so with
    e[j] = exp(sj[j] - max(sj)),  Wm = A with diag forced to 1,
    num  = Wm @ (agg * e[:,None]),  den = Wm @ e
    out  = relu(num / den[:,None] + b)        (b enters additively at the end)
No NxN intermediate is ever materialized.

Device work: two SPMD launches over 8 NeuronCores, row-sharded (1024 rows
per core). Between launches the host gathers agg/sj (1 MB), computes
e = exp(sj - max sj) and re-shards G = [agg*e | e].

A^T slices are prepared on the host so the contraction index lands on
SBUF partitions with no on-device transposes of A.

Launch 1 (agg + sj): A is binary {0,1}, so it ships as fp8e4m3 EXACTLY
(half of bf16 bytes). Y = X@W is computed on device in f32 and expanded
into a scaled 3-level fp8 split (y ~= q0 + q1/16 + q2/256,
q_k = fp8(16^k r_k)), recovering ~2^-20 relative accuracy from fp8
matmuls — accuracy matters most for sj since it enters an exponent.
Matmuls run in DoubleRow perf mode (2 fp8 k-chunks per instruction);
the two PSUM chains hold split levels (0,1) and (2) and the host
recombines them with 16^-k scales and derives sj = phi_j^T agg.

Launch 2 (masked weighted average): G must stay bf16 — e's dynamic range
(down to ~e^-80 of the max) far exceeds fp8's exponent range, and rows
whose neighborhoods sit deep below the global max would flush to 0/0.
A and G ride one bf16 array whose rows are [A^T[j,:] | G[j,:]], so the
small G stationary tiles come in on the big A DMA with full-size
descriptors. The forced mask diagonal enters via the host-prepared GdT
term (f32), which also guarantees den > 0.
"""

import numpy as np
import ml_dtypes

import concourse.bass as bass
from concourse import bacc
import concourse.mybir as mybir
import concourse.tile as tile
from concourse.bass_utils import run_bass_kernel_spmd
from concourse.masks import make_identity
from contextlib import ExitStack

F32 = mybir.dt.float32
FP8 = mybir.dt.float8e4
F8 = ml_dtypes.float8_e4m3
BF16 = mybir.dt.bfloat16
DR = mybir.MatmulPerfMode.DoubleRow

N = 8192
F_IN = 128
F_OUT = 64
CORES = 8
NL = N // CORES  # local rows per core
P = 128
GRP = 16  # j-chunks per A DMA
YGRP = 8  # j-chunks per Y-split batch

_cache = {}


def _run(nc, in_maps, cores):
    import time

    last = None
    for attempt in range(3):
        try:
            return run_bass_kernel_spmd(nc, in_maps, cores).results
        except Exception as exc:  # transient NRT/axon worker hiccups
            last = exc
            time.sleep(5 * (attempt + 1))
    raise last


def _build_launch1(n, nl, f_in, f_out):
    """Per core: Y = X@W, scaled 3-level fp8 split of Y, two DoubleRow
    accumulation chains (levels 0,1 and 2) of (A_loc @ Y)^T.
    Outputs o1 = [chainA (128 rows); chainB (64 rows)]."""
    njc = n // P
    nc = bacc.Bacc(None, target_bir_lowering=False)
    at = nc.dram_tensor("at", [n, nl], FP8, kind="ExternalInput")
    xt = nc.dram_tensor("xt", [f_in, n], F32, kind="ExternalInput")
    w = nc.dram_tensor("w", [f_in, f_out], F32, kind="ExternalInput")
    # rows 0:128 chainA (levels 0,1), 128:192 chainB (level 2);
    # the host recombines with 16^-k scales and derives sj = phi_j^T agg
    o1 = nc.dram_tensor("o1", [P + f_out, nl], F32, kind="ExternalOutput")

    ngrp = njc // GRP

    with tile.TileContext(nc) as tc, ExitStack() as ctx:
        singles = ctx.enter_context(tc.tile_pool(name="singles", bufs=1))
        at_pool = ctx.enter_context(tc.tile_pool(name="at", bufs=3))
        sp = ctx.enter_context(tc.tile_pool(name="split", bufs=2))
        ps_y = ctx.enter_context(tc.tile_pool(name="psy", bufs=2, space="PSUM"))
        ps_big = ctx.enter_context(tc.tile_pool(name="psbig", bufs=1, space="PSUM"))

        w_sb = singles.tile([f_in, f_out], F32)
        nc.sync.dma_start(out=w_sb, in_=w[:, :])
        xt_sb = singles.tile([f_in, n], F32)
        nxc = n // 8
        for xc in range(8):
            xeng = nc.sync if xc % 2 == 0 else nc.gpsimd
            xeng.dma_start(
                out=xt_sb[:, xc * nxc : (xc + 1) * nxc],
                in_=xt[:, xc * nxc : (xc + 1) * nxc],
            )

        # fp8 stationary splits: ysA = [q0 | q1], ysB = [q2] per j-chunk
        ysA = singles.tile([P, njc, 2 * f_out], FP8)
        ysB = singles.tile([P, njc, f_out], FP8)

        fo = f_out
        # scaled fp8 split: q0=fp8(y); r=y-q0; q1=fp8(16r); r2=16r-q1;
        # q2=fp8(16r2). Emitted as a software-pipelined wavefront: engine
        # queues are strict in-order, so group-by-group emission would make
        # each ACT cast head-of-line-block behind one still waiting on DVE.
        Copy = mybir.ActivationFunctionType.Copy
        ngroups = njc // YGRP
        yps_g, r_g = {}, {}
        for g in range(ngroups + 2):
            if g < ngroups:
                yps = ps_y.tile([P, YGRP, fo], F32)
                for k in range(YGRP):
                    jc = g * YGRP + k
                    nc.tensor.matmul(
                        yps[:, k, :],
                        xt_sb[:, jc * P : (jc + 1) * P],
                        w_sb[:],
                        start=True,
                        stop=True,
                    )
                yps_g[g] = yps
                s = slice(g * YGRP, (g + 1) * YGRP)
                nc.vector.tensor_copy(ysA[:, s, 0:fo], yps[:])
            if 0 <= g - 1 < ngroups:
                gg = g - 1
                s = slice(gg * YGRP, (gg + 1) * YGRP)
                r = sp.tile([P, YGRP, fo], F32)
                nc.vector.tensor_sub(r[:], yps_g[gg][:], ysA[:, s, 0:fo])
                nc.scalar.activation(ysA[:, s, fo : 2 * fo], r[:], Copy, scale=16.0)
                r_g[gg] = r
            if 0 <= g - 2 < ngroups:
                gg = g - 2
                s = slice(gg * YGRP, (gg + 1) * YGRP)
                r2 = sp.tile([P, YGRP, fo], F32)
                nc.vector.scalar_tensor_tensor(
                    r2[:],
                    r_g.pop(gg)[:],
                    16.0,
                    ysA[:, s, fo : 2 * fo],
                    mybir.AluOpType.mult,
                    mybir.AluOpType.subtract,
                )
                nc.scalar.activation(ysB[:, s, 0:fo], r2[:], Copy, scale=16.0)

        # pass 1: two DoubleRow chains, psum rows = split levels x f_out
        pA = ps_big.tile([P, nl], F32)
        pB = ps_big.tile([f_out, nl], F32)
        nh = nl // 512 if nl >= 512 else 1
        hw = min(nl, 512)
        at_r = at.rearrange("(a g p) i -> a p g i", a=ngrp, p=P)
        for a in range(ngrp):
            at_sb = at_pool.tile([P, GRP, nl], FP8)
            eng = nc.sync if a % 2 == 0 else nc.gpsimd
            eng.dma_start(out=at_sb, in_=at_r[a])
            for kp in range(GRP // 2):
                jc = a * GRP + kp * 2
                for h in range(nh):
                    for ps, ys in ((pA, ysA), (pB, ysB)):
                        nc.tensor.matmul(
                            ps[:, h * hw : (h + 1) * hw],
                            ys[:, jc : jc + 2, :],
                            at_sb[:, kp * 2 : kp * 2 + 2, h * hw : (h + 1) * hw],
                            start=(jc == 0),
                            stop=(jc == njc - 2),
                            perf_mode=DR,
                        )

        a2A = singles.tile([P, nl], F32)
        nc.vector.tensor_copy(a2A[:], pA[:])
        a2B = singles.tile([f_out, nl], F32)
        nc.scalar.activation(
            a2B[:], pB[:], mybir.ActivationFunctionType.Copy
        )

        nc.sync.dma_start(out=o1[0:P, :], in_=a2A[:])
        nc.scalar.dma_start(out=o1[P : P + f_out, :], in_=a2B[:])
    nc.finalize()
    return nc


def _build_launch2(n, nl, f_out, has_bias):
    """Per core: R = Wm_loc @ G + Gd, out = relu(num/den).

    A and G ride one bf16 array whose rows are [A^T[j, :] | G[j, :]] — the G
    tiles come in on the big A DMA with full-size descriptors. G must stay
    bf16 (not fp8 splits): e's dynamic range (down to ~e^-80) far exceeds
    fp8's exponent range.

    The A tile is the STATIONARY operand ([128, 128], full PE array) and G
    the moving one ([128, 65]) — half the PE row-cycles of the reverse
    orientation, and the result lands in natural [i, f] layout so the
    epilogue needs no PE transposes. Eight [128, 65] PSUM accumulators
    (one per local 128-row block) use all 8 banks."""
    njc = n // P
    fe = f_out + 1
    nic = nl // P
    nc = bacc.Bacc(None, target_bir_lowering=False)
    atg = nc.dram_tensor("atg", [n, nl + fe], BF16, kind="ExternalInput")
    # diag fix in natural layout: gdn[i, :] = (1 - A[ii]) * G[i, :]
    gdn = nc.dram_tensor("gdn", [nl, fe], F32, kind="ExternalInput")
    if has_bias:
        be = nc.dram_tensor("be", [1, f_out], F32, kind="ExternalInput")
    out = nc.dram_tensor("out", [nl, f_out], F32, kind="ExternalOutput")

    with tile.TileContext(nc) as tc, ExitStack() as ctx:
        singles = ctx.enter_context(tc.tile_pool(name="singles", bufs=1))
        at_pool = ctx.enter_context(tc.tile_pool(name="at", bufs=5))
        h_pool = ctx.enter_context(tc.tile_pool(name="h", bufs=3))
        ps_out = ctx.enter_context(tc.tile_pool(name="pso", bufs=1, space="PSUM"))

        gdn_sb = singles.tile([P, nic, fe], F32)
        nc.scalar.dma_start(out=gdn_sb, in_=gdn.rearrange("(g p) f -> p g f", p=P))
        if has_bias:
            # bias broadcast across partitions for the per-row rank-1 add
            bb_sb = singles.tile([P, f_out], F32)
            nc.sync.dma_start(out=bb_sb, in_=be[:, :].to_broadcast((P, f_out)))

        accs = [
            ps_out.tile([P, fe], F32, tag=f"acc{ic}", name=f"acc{ic}")
            for ic in range(nic)
        ]
        # small leading groups so PE starts within ~2.5us of launch
        sizes = [4, 8, 8, 8, 8, 8, 8, 8, 4] if njc >= 16 else [njc]
        gmax = max(sizes)
        js = 0
        engs = [nc.sync, nc.scalar, nc.gpsimd]
        for a, gsz in enumerate(sizes):
            at_sb = at_pool.tile([P, gmax, nl + fe], BF16)
            engs[a % 3].dma_start(
                out=at_sb[:, 0:gsz, :],
                in_=atg[js * P : (js + gsz) * P, :].rearrange(
                    "(g p) i -> p g i", p=P
                ),
            )
            for k in range(gsz):
                jc = js + k
                for ic in range(nic):
                    nc.tensor.matmul(
                        accs[ic][:],
                        at_sb[:, k, ic * P : (ic + 1) * P],
                        at_sb[:, k, nl : nl + fe],
                        start=(jc == 0),
                        stop=(jc == njc - 1),
                    )
            js += gsz

        # epilogue per 128-row block: add diag term, out = relu(num/den)
        for ic in range(nic):
            su = h_pool.tile([P, fe], F32)
            nc.vector.tensor_add(su[:], accs[ic][:], gdn_sb[:, ic, :])
            if has_bias:
                su2 = h_pool.tile([P, fe], F32)
                # num += bias * den (per-partition scalar = den column)
                nc.vector.scalar_tensor_tensor(
                    su2[:, 0:f_out],
                    bb_sb[:],
                    su[:, f_out : f_out + 1],
                    su[:, 0:f_out],
                    mybir.AluOpType.mult,
                    mybir.AluOpType.add,
                )
                nc.vector.tensor_copy(su2[:, f_out : f_out + 1], su[:, f_out : f_out + 1])
                su = su2
            rec = h_pool.tile([P, 1], F32)
            nc.vector.reciprocal(rec[:], su[:, f_out : f_out + 1])
            h_sb = h_pool.tile([P, f_out], F32)
            nc.scalar.activation(
                h_sb[:],
                su[:, 0:f_out],
                mybir.ActivationFunctionType.Relu,
                scale=rec[:],
            )
            eng = nc.sync if ic % 2 == 0 else nc.scalar
            eng.dma_start(out=out[ic * P : (ic + 1) * P, :], in_=h_sb[:])
    nc.finalize()
    return nc


def _get_programs(has_bias):
    key = (N, NL, F_IN, F_OUT, has_bias)
    if key not in _cache:
        _cache[key] = (
            _build_launch1(N, NL, F_IN, F_OUT),
            _build_launch2(N, NL, F_OUT, has_bias),
        )
    return _cache[key]


def kernel(A, X, weight, bias, phi):
    A = np.asarray(A, dtype=np.float32)
    X = np.asarray(X, dtype=np.float32)
    weight = np.asarray(weight, dtype=np.float32)
    bias = np.asarray(bias, dtype=np.float32)
    phi = np.asarray(phi, dtype=np.float32)

    has_bias = bool(np.any(bias))
    nc1, nc2 = _get_programs(has_bias)
    cores = list(range(CORES))

    # host-side sharding / layout prep (A is {0,1}: fp8 cast is exact)
    at_slices = [
        np.ascontiguousarray(A[c * NL : (c + 1) * NL, :].astype(F8).T)
        for c in range(CORES)
    ]
    xt = np.ascontiguousarray(X.T)
    pj = phi[F_OUT:, 0]

    in1 = [{"at": at_slices[c], "xt": xt, "w": weight} for c in range(CORES)]
    res1 = _run(nc1, in1, cores)

    # host glue: reassemble agg from scaled split chains, compute e and G
    scales = np.array([1.0, 1 / 16.0, 1 / 256.0])[:, None, None]
    aggT = np.concatenate(
        [
            (res1[c]["o1"].reshape(3, F_OUT, NL) * scales).sum(axis=0)
            for c in range(CORES)
        ],
        axis=1,
    )
    sj = (pj.astype(np.float64) @ aggT.astype(np.float64)).astype(np.float64)
    agg = np.ascontiguousarray(aggT.T)  # [N, F_OUT] f32, no bias
    e = np.exp(sj - sj.max()).astype(np.float32)
    Gf = np.concatenate([agg * e[:, None], e[:, None]], axis=1)  # [N, fe] f32
    Gbf = Gf.astype(ml_dtypes.bfloat16)
    dvec = 1.0 - np.ascontiguousarray(np.diagonal(A)).astype(np.float32)

    in2 = []
    for c in range(CORES):
        gd = dvec[c * NL : (c + 1) * NL, None] * Gf[c * NL : (c + 1) * NL, :]
        m = {
            "atg": np.concatenate([at_slices[c].astype(ml_dtypes.bfloat16), Gbf], axis=1),
            "gdn": np.ascontiguousarray(gd),
        }
        if has_bias:
            m["be"] = bias.astype(np.float32)[None, :]
        in2.append(m)
    res2 = _run(nc2, in2, cores)

    out = np.concatenate([res2[c]["out"] for c in range(CORES)], axis=0)
    return out.astype(np.float32)


# revision 61
# speedup vs baseline: 1.0070x; 1.0070x over previous
"""GAT layer kernel for Trainium2, 8-core row-parallel SPMD.

Math (reference):
    agg  = (A @ X) @ W + b
    si   = agg @ phi[:F];  sj = agg @ phi[F:]
    H    = si[:,None] + sj[None,:];  mask = (A + I) != 0
    attn = softmax(where(mask, H, -inf), axis=-1)
    out  = relu(attn @ agg)

Key identity: si[i] cancels in the row softmax, so with
    e[j] = exp(sj[j] - max(sj)),  Wm = A with diag forced to 1,
    num  = Wm @ (agg * e[:,None]),  den = Wm @ e
    out  = relu(num / den[:,None] + b)        (b enters additively at the end)
No NxN intermediate is ever materialized.

Device work: two SPMD launches over 8 NeuronCores, row-sharded (1024 rows
per core). Between launches the host gathers agg/sj (1 MB), computes
e = exp(sj - max sj) and re-shards G = [agg*e | e].

A^T slices are prepared on the host so the contraction index lands on
SBUF partitions with no on-device transposes of A.

Launch 1 (agg + sj): A is binary {0,1}, so it ships as fp8e4m3 EXACTLY
(half of bf16 bytes). Y = X@W is computed on device in f32 and expanded
into a scaled 3-level fp8 split (y ~= q0 + q1/16 + q2/256,
q_k = fp8(16^k r_k)), recovering ~2^-20 relative accuracy from fp8
matmuls — accuracy matters most for sj since it enters an exponent.
Matmuls run in DoubleRow perf mode (2 fp8 k-chunks per instruction);
the two PSUM chains hold split levels (0,1) and (2) and the host
recombines them with 16^-k scales and derives sj = phi_j^T agg.

Launch 2 (masked weighted average): G must stay bf16 — e's dynamic range
(down to ~e^-80 of the max) far exceeds fp8's exponent range, and rows
whose neighborhoods sit deep below the global max would flush to 0/0.
A and G ride one bf16 array whose rows are [Wm^T[j,:] | G[j,:]] (mask
diagonal pre-forced to 1 on the host, bf16-exact), so the small G tiles
come in on the big A DMA with full-size descriptors and no separate
diag-fix term is needed.
"""

import numpy as np
import ml_dtypes

import concourse.bass as bass
from concourse import bacc
import concourse.mybir as mybir
import concourse.tile as tile
from concourse.bass_utils import run_bass_kernel_spmd
from concourse.masks import make_identity
from contextlib import ExitStack

F32 = mybir.dt.float32
FP8 = mybir.dt.float8e4
F8 = ml_dtypes.float8_e4m3
BF16 = mybir.dt.bfloat16
DR = mybir.MatmulPerfMode.DoubleRow

N = 8192
F_IN = 128
F_OUT = 64
CORES = 8
NL = N // CORES  # local rows per core
P = 128
GRP = 16  # j-chunks per A DMA
YGRP = 8  # j-chunks per Y-split batch

_cache = {}


def _run(nc, in_maps, cores):
    import time

    last = None
    for attempt in range(3):
        try:
            return run_bass_kernel_spmd(nc, in_maps, cores).results
        except Exception as exc:  # transient NRT/axon worker hiccups
            last = exc
            time.sleep(5 * (attempt + 1))
    raise last


def _build_launch1(n, nl, f_in, f_out):
    """Per core: Y = X@W, scaled 3-level fp8 split of Y, two DoubleRow
    accumulation chains (levels 0,1 and 2) of (A_loc @ Y)^T.
    Outputs o1 = [chainA (128 rows); chainB (64 rows)]."""
    njc = n // P
    nc = bacc.Bacc(None, target_bir_lowering=False)
    at = nc.dram_tensor("at", [n, nl], FP8, kind="ExternalInput")
    xt = nc.dram_tensor("xt", [f_in, n], F32, kind="ExternalInput")
    w = nc.dram_tensor("w", [f_in, f_out], F32, kind="ExternalInput")
    # rows 0:128 chainA (levels 0,1), 128:192 chainB (level 2);
    # the host recombines with 16^-k scales and derives sj = phi_j^T agg
    o1 = nc.dram_tensor("o1", [P + f_out, nl], F32, kind="ExternalOutput")

    ngrp = njc // GRP

    with tile.TileContext(nc) as tc, ExitStack() as ctx:
        singles = ctx.enter_context(tc.tile_pool(name="singles", bufs=1))
        at_pool = ctx.enter_context(tc.tile_pool(name="at", bufs=3))
        sp = ctx.enter_context(tc.tile_pool(name="split", bufs=2))
        ps_y = ctx.enter_context(tc.tile_pool(name="psy", bufs=2, space="PSUM"))
        ps_big = ctx.enter_context(tc.tile_pool(name="psbig", bufs=1, space="PSUM"))

        w_sb = singles.tile([f_in, f_out], F32)
        nc.sync.dma_start(out=w_sb, in_=w[:, :])
        xt_sb = singles.tile([f_in, n], F32)
        nxc = n // 8
        for xc in range(8):
            xeng = nc.sync if xc % 2 == 0 else nc.gpsimd
            xeng.dma_start(
                out=xt_sb[:, xc * nxc : (xc + 1) * nxc],
                in_=xt[:, xc * nxc : (xc + 1) * nxc],
            )

        # fp8 stationary splits: ysA = [q0 | q1], ysB = [q2] per j-chunk
        ysA = singles.tile([P, njc, 2 * f_out], FP8)
        ysB = singles.tile([P, njc, f_out], FP8)

        fo = f_out
        # scaled fp8 split: q0=fp8(y); r=y-q0; q1=fp8(16r); r2=16r-q1;
        # q2=fp8(16r2). Emitted as a software-pipelined wavefront: engine
        # queues are strict in-order, so group-by-group emission would make
        # each ACT cast head-of-line-block behind one still waiting on DVE.
        Copy = mybir.ActivationFunctionType.Copy
        ngroups = njc // YGRP
        yps_g, r_g = {}, {}
        for g in range(ngroups + 2):
            if g < ngroups:
                yps = ps_y.tile([P, YGRP, fo], F32)
                for k in range(YGRP):
                    jc = g * YGRP + k
                    nc.tensor.matmul(
                        yps[:, k, :],
                        xt_sb[:, jc * P : (jc + 1) * P],
                        w_sb[:],
                        start=True,
                        stop=True,
                    )
                yps_g[g] = yps
                s = slice(g * YGRP, (g + 1) * YGRP)
                nc.vector.tensor_copy(ysA[:, s, 0:fo], yps[:])
            if 0 <= g - 1 < ngroups:
                gg = g - 1
                s = slice(gg * YGRP, (gg + 1) * YGRP)
                r = sp.tile([P, YGRP, fo], F32)
                nc.vector.tensor_sub(r[:], yps_g[gg][:], ysA[:, s, 0:fo])
                nc.scalar.activation(ysA[:, s, fo : 2 * fo], r[:], Copy, scale=16.0)
                r_g[gg] = r
            if 0 <= g - 2 < ngroups:
                gg = g - 2
                s = slice(gg * YGRP, (gg + 1) * YGRP)
                r2 = sp.tile([P, YGRP, fo], F32)
                nc.vector.scalar_tensor_tensor(
                    r2[:],
                    r_g.pop(gg)[:],
                    16.0,
                    ysA[:, s, fo : 2 * fo],
                    mybir.AluOpType.mult,
                    mybir.AluOpType.subtract,
                )
                nc.scalar.activation(ysB[:, s, 0:fo], r2[:], Copy, scale=16.0)

        # pass 1: two DoubleRow chains, psum rows = split levels x f_out
        pA = ps_big.tile([P, nl], F32)
        pB = ps_big.tile([f_out, nl], F32)
        nh = nl // 512 if nl >= 512 else 1
        hw = min(nl, 512)
        at_r = at.rearrange("(a g p) i -> a p g i", a=ngrp, p=P)
        for a in range(ngrp):
            at_sb = at_pool.tile([P, GRP, nl], FP8)
            eng = nc.sync if a % 2 == 0 else nc.gpsimd
            eng.dma_start(out=at_sb, in_=at_r[a])
            for kp in range(GRP // 2):
                jc = a * GRP + kp * 2
                for h in range(nh):
                    for ps, ys in ((pA, ysA), (pB, ysB)):
                        nc.tensor.matmul(
                            ps[:, h * hw : (h + 1) * hw],
                            ys[:, jc : jc + 2, :],
                            at_sb[:, kp * 2 : kp * 2 + 2, h * hw : (h + 1) * hw],
                            start=(jc == 0),
                            stop=(jc == njc - 2),
                            perf_mode=DR,
                        )

        a2A = singles.tile([P, nl], F32)
        nc.vector.tensor_copy(a2A[:], pA[:])
        a2B = singles.tile([f_out, nl], F32)
        nc.scalar.activation(
            a2B[:], pB[:], mybir.ActivationFunctionType.Copy
        )

        nc.sync.dma_start(out=o1[0:P, :], in_=a2A[:])
        nc.scalar.dma_start(out=o1[P : P + f_out, :], in_=a2B[:])
    nc.finalize()
    return nc


def _build_launch2(n, nl, f_out, has_bias):
    """Per core: R = Wm_loc @ G + Gd, out = relu(num/den).

    A and G ride one bf16 array whose rows are [A^T[j, :] | G[j, :]] — the G
    tiles come in on the big A DMA with full-size descriptors. G must stay
    bf16 (not fp8 splits): e's dynamic range (down to ~e^-80) far exceeds
    fp8's exponent range.

    The A tile is the STATIONARY operand ([128, 128], full PE array) and G
    the moving one ([128, 65]) — half the PE row-cycles of the reverse
    orientation, and the result lands in natural [i, f] layout so the
    epilogue needs no PE transposes. Eight [128, 65] PSUM accumulators
    (one per local 128-row block) use all 8 banks."""
    njc = n // P
    fe = f_out + 1
    nic = nl // P
    nc = bacc.Bacc(None, target_bir_lowering=False)
    # rows are [Wm^T[j, :] | G[j, :]] with the mask diagonal pre-forced to 1
    # on the host (bf16-exact), so no separate diag-fix term is needed
    atg = nc.dram_tensor("atg", [n, nl + fe], BF16, kind="ExternalInput")
    if has_bias:
        be = nc.dram_tensor("be", [1, f_out], F32, kind="ExternalInput")
    out = nc.dram_tensor("out", [nl, f_out], F32, kind="ExternalOutput")

    with tile.TileContext(nc) as tc, ExitStack() as ctx:
        singles = ctx.enter_context(tc.tile_pool(name="singles", bufs=1))
        at_pool = ctx.enter_context(tc.tile_pool(name="at", bufs=5))
        h_pool = ctx.enter_context(tc.tile_pool(name="h", bufs=3))
        ps_out = ctx.enter_context(tc.tile_pool(name="pso", bufs=1, space="PSUM"))

        if has_bias:
            # bias broadcast across partitions for the per-row rank-1 add
            bb_sb = singles.tile([P, f_out], F32)
            nc.sync.dma_start(out=bb_sb, in_=be[:, :].to_broadcast((P, f_out)))

        accs = [
            ps_out.tile([P, fe], F32, tag=f"acc{ic}", name=f"acc{ic}")
            for ic in range(nic)
        ]
        # small leading groups so PE starts within ~2.5us of launch
        sizes = [4, 8, 8, 8, 8, 8, 8, 8, 4] if njc >= 16 else [njc]
        gmax = max(sizes)
        js = 0
        engs = [nc.sync, nc.scalar, nc.gpsimd]
        for a, gsz in enumerate(sizes):
            at_sb = at_pool.tile([P, gmax, nl + fe], BF16)
            engs[a % 3].dma_start(
                out=at_sb[:, 0:gsz, :],
                in_=atg[js * P : (js + gsz) * P, :].rearrange(
                    "(g p) i -> p g i", p=P
                ),
            )
            for k in range(gsz):
                jc = js + k
                for ic in range(nic):
                    nc.tensor.matmul(
                        accs[ic][:],
                        at_sb[:, k, ic * P : (ic + 1) * P],
                        at_sb[:, k, nl : nl + fe],
                        start=(jc == 0),
                        stop=(jc == njc - 1),
                    )
            js += gsz

        # epilogue per 128-row block: out = relu(num/den) straight from PSUM
        for ic in range(nic):
            su = accs[ic]
            if has_bias:
                su2 = h_pool.tile([P, fe], F32)
                # num += bias * den (per-partition scalar = den column)
                nc.vector.scalar_tensor_tensor(
                    su2[:, 0:f_out],
                    bb_sb[:],
                    su[:, f_out : f_out + 1],
                    su[:, 0:f_out],
                    mybir.AluOpType.mult,
                    mybir.AluOpType.add,
                )
                nc.vector.tensor_copy(su2[:, f_out : f_out + 1], su[:, f_out : f_out + 1])
                su = su2
            rec = h_pool.tile([P, 1], F32)
            nc.vector.reciprocal(rec[:], su[:, f_out : f_out + 1])
            h_sb = h_pool.tile([P, f_out], F32)
            nc.scalar.activation(
                h_sb[:],
                su[:, 0:f_out],
                mybir.ActivationFunctionType.Relu,
                scale=rec[:],
            )
            eng = nc.sync if ic % 2 == 0 else nc.scalar
            eng.dma_start(out=out[ic * P : (ic + 1) * P, :], in_=h_sb[:])
    nc.finalize()
    return nc


def _get_programs(has_bias):
    key = (N, NL, F_IN, F_OUT, has_bias)
    if key not in _cache:
        _cache[key] = (
            _build_launch1(N, NL, F_IN, F_OUT),
            _build_launch2(N, NL, F_OUT, has_bias),
        )
    return _cache[key]


def kernel(A, X, weight, bias, phi):
    A = np.asarray(A, dtype=np.float32)
    X = np.asarray(X, dtype=np.float32)
    weight = np.asarray(weight, dtype=np.float32)
    bias = np.asarray(bias, dtype=np.float32)
    phi = np.asarray(phi, dtype=np.float32)

    has_bias = bool(np.any(bias))
    nc1, nc2 = _get_programs(has_bias)
    cores = list(range(CORES))

    # host-side sharding / layout prep (A is {0,1}: fp8 cast is exact)
    at_slices = [
        np.ascontiguousarray(A[c * NL : (c + 1) * NL, :].astype(F8).T)
        for c in range(CORES)
    ]
    xt = np.ascontiguousarray(X.T)
    pj = phi[F_OUT:, 0]

    in1 = [{"at": at_slices[c], "xt": xt, "w": weight} for c in range(CORES)]
    res1 = _run(nc1, in1, cores)

    # host glue: reassemble agg from scaled split chains, compute e and G
    scales = np.array([1.0, 1 / 16.0, 1 / 256.0])[:, None, None]
    aggT = np.concatenate(
        [
            (res1[c]["o1"].reshape(3, F_OUT, NL) * scales).sum(axis=0)
            for c in range(CORES)
        ],
        axis=1,
    )
    sj = (pj.astype(np.float64) @ aggT.astype(np.float64)).astype(np.float64)
    agg = np.ascontiguousarray(aggT.T)  # [N, F_OUT] f32, no bias
    e = np.exp(sj - sj.max()).astype(np.float32)
    Gf = np.concatenate([agg * e[:, None], e[:, None]], axis=1)  # [N, fe] f32
    Gbf = Gf.astype(ml_dtypes.bfloat16)

    il = np.arange(NL)
    in2 = []
    for c in range(CORES):
        am = at_slices[c].astype(ml_dtypes.bfloat16)
        am[c * NL + il, il] = ml_dtypes.bfloat16(1.0)  # Wm diag = 1, exact
        m = {"atg": np.concatenate([am, Gbf], axis=1)}
        if has_bias:
            m["be"] = bias.astype(np.float32)[None, :]
        in2.append(m)
    res2 = _run(nc2, in2, cores)

    out = np.concatenate([res2[c]["out"] for c in range(CORES)], axis=0)
    return out.astype(np.float32)


# revision 62
# speedup vs baseline: 1.1091x; 1.1014x over previous
"""GAT layer kernel for Trainium2, 8-core row-parallel SPMD.

Math (reference):
    agg  = (A @ X) @ W + b
    si   = agg @ phi[:F];  sj = agg @ phi[F:]
    H    = si[:,None] + sj[None,:];  mask = (A + I) != 0
    attn = softmax(where(mask, H, -inf), axis=-1)
    out  = relu(attn @ agg)

Key identity: si[i] cancels in the row softmax, so with
    e[j] = exp(sj[j] - max(sj)),  Wm = A with diag forced to 1,
    num  = Wm @ (agg * e[:,None]),  den = Wm @ e
    out  = relu(num / den[:,None] + b)        (b enters additively at the end)
No NxN intermediate is ever materialized.

Device work: two SPMD launches over 8 NeuronCores, row-sharded (1024 rows
per core). Between launches the host gathers agg/sj (1 MB), computes
e = exp(sj - max sj) and re-shards G = [agg*e | e].

A^T slices are prepared on the host so the contraction index lands on
SBUF partitions with no on-device transposes of A.

Launch 1 (agg + sj): A is binary {0,1}, so it ships as fp8e4m3 EXACTLY
(half of bf16 bytes). Y = X@W is computed on device in f32 and expanded
into a scaled 3-level fp8 split (y ~= q0 + q1/16 + q2/256,
q_k = fp8(16^k r_k)), recovering ~2^-20 relative accuracy from fp8
matmuls — accuracy matters most for sj since it enters an exponent.
Matmuls run in DoubleRow perf mode (2 fp8 k-chunks per instruction);
the two PSUM chains hold split levels (0,1) and (2) and the host
recombines them with 16^-k scales and derives sj = phi_j^T agg.

Launch 2 (masked weighted average): G must stay bf16 — e's dynamic range
(down to ~e^-80 of the max) far exceeds fp8's exponent range, and rows
whose neighborhoods sit deep below the global max would flush to 0/0.
A and G ride one bf16 array whose rows are [Wm^T[j,:] | G[j,:]] (mask
diagonal pre-forced to 1 on the host, bf16-exact), so the small G tiles
come in on the big A DMA with full-size descriptors and no separate
diag-fix term is needed.
"""

import numpy as np
import ml_dtypes

import concourse.bass as bass
from concourse import bacc
import concourse.mybir as mybir
import concourse.tile as tile
from concourse.bass_utils import run_bass_kernel_spmd
from concourse.masks import make_identity
from contextlib import ExitStack

F32 = mybir.dt.float32
FP8 = mybir.dt.float8e4
F8 = ml_dtypes.float8_e4m3
BF16 = mybir.dt.bfloat16
DR = mybir.MatmulPerfMode.DoubleRow

N = 8192
F_IN = 128
F_OUT = 64
CORES = 8
NL = N // CORES  # local rows per core
P = 128
GRP = 16  # j-chunks per A DMA
YGRP = 8  # j-chunks per Y-split batch

_cache = {}


def _run(nc, in_maps, cores):
    import time

    last = None
    for attempt in range(3):
        try:
            return run_bass_kernel_spmd(nc, in_maps, cores).results
        except Exception as exc:  # transient NRT/axon worker hiccups
            last = exc
            time.sleep(5 * (attempt + 1))
    raise last


def _build_launch1(n, nl, f_in, f_out):
    """Per core: Y = X@W, scaled 3-level fp8 split of Y, two DoubleRow
    accumulation chains (levels 0,1 and 2) of (A_loc @ Y)^T.
    Outputs o1 = [chainA (128 rows); chainB (64 rows)]."""
    njc = n // P
    nc = bacc.Bacc(None, target_bir_lowering=False)
    at = nc.dram_tensor("at", [n, nl], FP8, kind="ExternalInput")
    xt = nc.dram_tensor("xt", [f_in, n], F32, kind="ExternalInput")
    w = nc.dram_tensor("w", [f_in, f_out], F32, kind="ExternalInput")
    # rows 0:128 chainA (levels 0,1), 128:192 chainB (level 2);
    # the host recombines with 16^-k scales and derives sj = phi_j^T agg
    o1 = nc.dram_tensor("o1", [P + f_out, nl], F32, kind="ExternalOutput")

    ngrp = njc // GRP

    with tile.TileContext(nc) as tc, ExitStack() as ctx:
        singles = ctx.enter_context(tc.tile_pool(name="singles", bufs=1))
        at_pool = ctx.enter_context(tc.tile_pool(name="at", bufs=3))
        sp = ctx.enter_context(tc.tile_pool(name="split", bufs=2))
        ps_y = ctx.enter_context(tc.tile_pool(name="psy", bufs=2, space="PSUM"))
        ps_big = ctx.enter_context(tc.tile_pool(name="psbig", bufs=1, space="PSUM"))

        w_sb = singles.tile([f_in, f_out], F32)
        nc.sync.dma_start(out=w_sb, in_=w[:, :])
        xt_sb = singles.tile([f_in, n], F32)
        nxc = n // 8
        for xc in range(8):
            xeng = (nc.sync, nc.gpsimd, nc.scalar, nc.sync, nc.gpsimd, nc.scalar, nc.sync, nc.gpsimd)[xc]
            xeng.dma_start(
                out=xt_sb[:, xc * nxc : (xc + 1) * nxc],
                in_=xt[:, xc * nxc : (xc + 1) * nxc],
            )

        # fp8 stationary splits: ysA = [q0 | q1], ysB = [q2] per j-chunk
        ysA = singles.tile([P, njc, 2 * f_out], FP8)
        ysB = singles.tile([P, njc, f_out], FP8)

        fo = f_out
        # scaled fp8 split: q0=fp8(y); r=y-q0; q1=fp8(16r); r2=16r-q1;
        # q2=fp8(16r2). Emitted as a software-pipelined wavefront: engine
        # queues are strict in-order, so group-by-group emission would make
        # each ACT cast head-of-line-block behind one still waiting on DVE.
        Copy = mybir.ActivationFunctionType.Copy
        ngroups = njc // YGRP
        yps_g, r_g = {}, {}
        for g in range(ngroups + 2):
            if g < ngroups:
                yps = ps_y.tile([P, YGRP, fo], F32)
                for k in range(YGRP):
                    jc = g * YGRP + k
                    nc.tensor.matmul(
                        yps[:, k, :],
                        xt_sb[:, jc * P : (jc + 1) * P],
                        w_sb[:],
                        start=True,
                        stop=True,
                    )
                yps_g[g] = yps
                s = slice(g * YGRP, (g + 1) * YGRP)
                nc.vector.tensor_copy(ysA[:, s, 0:fo], yps[:])
            if 0 <= g - 1 < ngroups:
                gg = g - 1
                s = slice(gg * YGRP, (gg + 1) * YGRP)
                r = sp.tile([P, YGRP, fo], F32)
                nc.vector.tensor_sub(r[:], yps_g[gg][:], ysA[:, s, 0:fo])
                nc.scalar.activation(ysA[:, s, fo : 2 * fo], r[:], Copy, scale=16.0)
                r_g[gg] = r
            if 0 <= g - 2 < ngroups:
                gg = g - 2
                s = slice(gg * YGRP, (gg + 1) * YGRP)
                r2 = sp.tile([P, YGRP, fo], F32)
                nc.vector.scalar_tensor_tensor(
                    r2[:],
                    r_g.pop(gg)[:],
                    16.0,
                    ysA[:, s, fo : 2 * fo],
                    mybir.AluOpType.mult,
                    mybir.AluOpType.subtract,
                )
                nc.scalar.activation(ysB[:, s, 0:fo], r2[:], Copy, scale=16.0)

        # pass 1: two DoubleRow chains, psum rows = split levels x f_out
        pA = ps_big.tile([P, nl], F32)
        pB = ps_big.tile([f_out, nl], F32)
        nh = nl // 512 if nl >= 512 else 1
        hw = min(nl, 512)
        at_r = at.rearrange("(a g p) i -> a p g i", a=ngrp, p=P)
        for a in range(ngrp):
            at_sb = at_pool.tile([P, GRP, nl], FP8)
            eng = nc.sync if a % 2 == 0 else nc.gpsimd
            eng.dma_start(out=at_sb, in_=at_r[a])
            for kp in range(GRP // 2):
                jc = a * GRP + kp * 2
                for h in range(nh):
                    for ps, ys in ((pA, ysA), (pB, ysB)):
                        nc.tensor.matmul(
                            ps[:, h * hw : (h + 1) * hw],
                            ys[:, jc : jc + 2, :],
                            at_sb[:, kp * 2 : kp * 2 + 2, h * hw : (h + 1) * hw],
                            start=(jc == 0),
                            stop=(jc == njc - 2),
                            perf_mode=DR,
                        )

        a2A = singles.tile([P, nl], F32)
        nc.vector.tensor_copy(a2A[:], pA[:])
        a2B = singles.tile([f_out, nl], F32)
        nc.scalar.activation(
            a2B[:], pB[:], mybir.ActivationFunctionType.Copy
        )

        nc.sync.dma_start(out=o1[0:P, :], in_=a2A[:])
        nc.scalar.dma_start(out=o1[P : P + f_out, :], in_=a2B[:])
    nc.finalize()
    return nc


def _build_launch2(n, nl, f_out, has_bias):
    """Per core: R = Wm_loc @ G + Gd, out = relu(num/den).

    A and G ride one bf16 array whose rows are [A^T[j, :] | G[j, :]] — the G
    tiles come in on the big A DMA with full-size descriptors. G must stay
    bf16 (not fp8 splits): e's dynamic range (down to ~e^-80) far exceeds
    fp8's exponent range.

    The A tile is the STATIONARY operand ([128, 128], full PE array) and G
    the moving one ([128, 65]) — half the PE row-cycles of the reverse
    orientation, and the result lands in natural [i, f] layout so the
    epilogue needs no PE transposes. Eight [128, 65] PSUM accumulators
    (one per local 128-row block) use all 8 banks."""
    njc = n // P
    fe = f_out + 1
    nic = nl // P
    nc = bacc.Bacc(None, target_bir_lowering=False)
    # rows are [Wm^T[j, :] | G[j, :]] with the mask diagonal pre-forced to 1
    # on the host (bf16-exact), so no separate diag-fix term is needed
    atg = nc.dram_tensor("atg", [n, nl + fe], BF16, kind="ExternalInput")
    if has_bias:
        be = nc.dram_tensor("be", [1, f_out], F32, kind="ExternalInput")
    out = nc.dram_tensor("out", [nl, f_out], F32, kind="ExternalOutput")

    with tile.TileContext(nc) as tc, ExitStack() as ctx:
        singles = ctx.enter_context(tc.tile_pool(name="singles", bufs=1))
        at_pool = ctx.enter_context(tc.tile_pool(name="at", bufs=5))
        h_pool = ctx.enter_context(tc.tile_pool(name="h", bufs=3))
        ps_out = ctx.enter_context(tc.tile_pool(name="pso", bufs=1, space="PSUM"))

        if has_bias:
            # bias broadcast across partitions for the per-row rank-1 add
            bb_sb = singles.tile([P, f_out], F32)
            nc.sync.dma_start(out=bb_sb, in_=be[:, :].to_broadcast((P, f_out)))

        accs = [
            ps_out.tile([P, fe], F32, tag=f"acc{ic}", name=f"acc{ic}")
            for ic in range(nic)
        ]
        # small leading groups so PE starts within ~2.5us of launch
        sizes = [4, 4, 8, 8, 8, 8, 8, 8, 8] if njc >= 16 else [njc]
        gmax = max(sizes)
        js = 0
        engs = [nc.sync, nc.scalar, nc.gpsimd]
        for a, gsz in enumerate(sizes):
            at_sb = at_pool.tile([P, gmax, nl + fe], BF16)
            engs[a % 3].dma_start(
                out=at_sb[:, 0:gsz, :],
                in_=atg[js * P : (js + gsz) * P, :].rearrange(
                    "(g p) i -> p g i", p=P
                ),
            )
            for k in range(gsz):
                jc = js + k
                for ic in range(nic):
                    nc.tensor.matmul(
                        accs[ic][:],
                        at_sb[:, k, ic * P : (ic + 1) * P],
                        at_sb[:, k, nl : nl + fe],
                        start=(jc == 0),
                        stop=(jc == njc - 1),
                    )
            js += gsz

        # epilogue per 128-row block: out = relu(num/den) straight from PSUM
        for ic in range(nic):
            su = accs[ic]
            if has_bias:
                su2 = h_pool.tile([P, fe], F32)
                # num += bias * den (per-partition scalar = den column)
                nc.vector.scalar_tensor_tensor(
                    su2[:, 0:f_out],
                    bb_sb[:],
                    su[:, f_out : f_out + 1],
                    su[:, 0:f_out],
                    mybir.AluOpType.mult,
                    mybir.AluOpType.add,
                )
                nc.vector.tensor_copy(su2[:, f_out : f_out + 1], su[:, f_out : f_out + 1])
                su = su2
            rec = h_pool.tile([P, 1], F32)
            nc.vector.reciprocal(rec[:], su[:, f_out : f_out + 1])
            h_sb = h_pool.tile([P, f_out], F32)
            nc.scalar.activation(
                h_sb[:],
                su[:, 0:f_out],
                mybir.ActivationFunctionType.Relu,
                scale=rec[:],
            )
            eng = nc.sync if ic % 2 == 0 else nc.scalar
            eng.dma_start(out=out[ic * P : (ic + 1) * P, :], in_=h_sb[:])
    nc.finalize()
    return nc


def _get_programs(has_bias):
    key = (N, NL, F_IN, F_OUT, has_bias)
    if key not in _cache:
        _cache[key] = (
            _build_launch1(N, NL, F_IN, F_OUT),
            _build_launch2(N, NL, F_OUT, has_bias),
        )
    return _cache[key]


def kernel(A, X, weight, bias, phi):
    A = np.asarray(A, dtype=np.float32)
    X = np.asarray(X, dtype=np.float32)
    weight = np.asarray(weight, dtype=np.float32)
    bias = np.asarray(bias, dtype=np.float32)
    phi = np.asarray(phi, dtype=np.float32)

    has_bias = bool(np.any(bias))
    nc1, nc2 = _get_programs(has_bias)
    cores = list(range(CORES))

    # host-side sharding / layout prep (A is {0,1}: fp8 cast is exact)
    at_slices = [
        np.ascontiguousarray(A[c * NL : (c + 1) * NL, :].astype(F8).T)
        for c in range(CORES)
    ]
    xt = np.ascontiguousarray(X.T)
    pj = phi[F_OUT:, 0]

    in1 = [{"at": at_slices[c], "xt": xt, "w": weight} for c in range(CORES)]
    res1 = _run(nc1, in1, cores)

    # host glue: reassemble agg from scaled split chains, compute e and G
    scales = np.array([1.0, 1 / 16.0, 1 / 256.0])[:, None, None]
    aggT = np.concatenate(
        [
            (res1[c]["o1"].reshape(3, F_OUT, NL) * scales).sum(axis=0)
            for c in range(CORES)
        ],
        axis=1,
    )
    sj = (pj.astype(np.float64) @ aggT.astype(np.float64)).astype(np.float64)
    agg = np.ascontiguousarray(aggT.T)  # [N, F_OUT] f32, no bias
    e = np.exp(sj - sj.max()).astype(np.float32)
    Gf = np.concatenate([agg * e[:, None], e[:, None]], axis=1)  # [N, fe] f32
    Gbf = Gf.astype(ml_dtypes.bfloat16)

    il = np.arange(NL)
    in2 = []
    for c in range(CORES):
        am = at_slices[c].astype(ml_dtypes.bfloat16)
        am[c * NL + il, il] = ml_dtypes.bfloat16(1.0)  # Wm diag = 1, exact
        m = {"atg": np.concatenate([am, Gbf], axis=1)}
        if has_bias:
            m["be"] = bias.astype(np.float32)[None, :]
        in2.append(m)
    res2 = _run(nc2, in2, cores)

    out = np.concatenate([res2[c]["out"] for c in range(CORES)], axis=0)
    return out.astype(np.float32)
